# revision 2
# baseline (speedup 1.0000x reference)
"""CharEmbeddingCNN Trainium2 kernel (fp8 DoubleRow one-hot formulation).

Reference computation (per word of L=20 chars):
    xe = emb[x]                       # [L, 256] -> treated as [256, L]
    y_k = conv1d_valid(xe, w_k) + b_k # k in (3,4,5), 256 -> 256 channels
    out = relu(max over all (k, t) of y_k[:, t]) * (len != 0)

Key idea: fold the embedding into the conv weights per tap:
    WTA_k,dk[c, o] = sum_i emb[c, i] * w_k[o, i, dk]      (alphabet tables)
    y_k[o, w, t]   = sum_dk WTA_k,dk[x[w, t+dk], o]
so the conv becomes a matmul of the alphabet tables against HOST-BUILT
one-hot character encodings. One-hot values (0/1) are exact in fp8, so
quantizing only WTA to e4m3 (scaled x64) keeps the output error ~1e-2.
Both operands fp8 enables perf_mode=DoubleRow: contraction 256 (the whole
alphabet, 2 k-tiles of 128) per instruction at 2 MACs/cell/cycle -- ~1.8x
the bf16 column rate, and no gpsimd gather stream at all.

Strategy (data-parallel over 8 NeuronCores, 1024 words each):
  - Host: one-hot oh[p, i, w*L+t] = (x[w,t] == i*128+p) as fp8 bytes,
    WTA packed [128, kdk, i, o] fp8, bias*64 f32, mask/64 f32.
  - Device: stream oh in word-chunks; per (ki, psum-bank group) run
    2*halves*k DoubleRow matmuls (dk-shifted one-hot views, both alphabet
    halves per instruction), then DVE segment-max over t into per-k
    accumulators M. Rounds interleave k=3/4/5 so combined coverage
    advances evenly; bias+3-way max combine, PE transpose to [word, ch],
    fused relu*(mask/64) on ScalarE, DMA out.
"""

import numpy as np
import ml_dtypes
from contextlib import ExitStack

import concourse.bacc as bacc
import concourse.tile as tile
from concourse import mybir
from concourse.bass_utils import run_bass_kernel_spmd

F32 = mybir.dt.float32
BF16 = mybir.dt.bfloat16
F8 = mybir.dt.float8e4

B, S, L = 64, 128, 20
EMB = 256
KS = (3, 4, 5)
NCORES = 8
W = (B * S) // NCORES          # words per core (1024)
NKDK = sum(KS)                 # 12 packed (k, dk) weight slices
SCALE = 64.0                   # WTA/bias scale so fp8 stays out of subnormals
WARMUP_MM = 22                 # dummy matmuls to warm the PE clock gate
OH_CHUNK = 64                  # words per input-DMA chunk


def _kdk_off(ki, dk):
    return sum(KS[:ki]) + dk


def _bank_plan(words):
    """Per-k list of (w0, halves, nw) PSUM-bank word groups. Each group is
    `halves` independent DoubleRow matmul chains of nw words x lk positions
    (nw*lk <= 256, the max moving free dim / 2), sharing one PSUM bank so a
    single reduce_max drains halves*nw words."""
    plans = []
    for ki, k in enumerate(KS):
        lk = L - k + 1
        nw = 256 // lk          # 14, 15, 16
        banks = []
        w0 = 0
        while w0 + 2 * nw <= words:
            banks.append((w0, 2, nw))
            w0 += 2 * nw
        rem = words - w0
        if rem:
            if rem * lk <= 256:
                banks.append((w0, 1, rem))
            else:
                assert rem % 2 == 0 and (rem // 2) * lk <= 256
                banks.append((w0, 2, rem // 2))
        plans.append(banks)
    return plans


def build_bass(words=W):
    nwb = words // 128          # output word-blocks of 128
    plans = _bank_plan(words)

    nc = bacc.Bacc(
        "TRN2",
        target_bir_lowering=False,
        debug=False,
        enable_asserts=False,
        num_swdge_queues=1,
    )

    oh_d = nc.dram_tensor("oh", [128, 2, words * L], F8,
                          kind="ExternalInput").ap()
    wta_d = nc.dram_tensor("wta", [128, NKDK * 2 * EMB], F8,
                           kind="ExternalInput").ap()
    bias_d = nc.dram_tensor("bias", [128, 6], F32, kind="ExternalInput").ap()
    mask_d = nc.dram_tensor("maskp", [128, nwb], F32, kind="ExternalInput").ap()
    id_d = nc.dram_tensor("ident", [128, 128], F32, kind="ExternalInput").ap()
    out_d = nc.dram_tensor("out", [words, EMB], F32, kind="ExternalOutput").ap()

    with tile.TileContext(nc) as tc, ExitStack() as ctx:
        const_pool = ctx.enter_context(tc.tile_pool(name="const", bufs=1))
        psum_pool = ctx.enter_context(tc.tile_pool(name="ps", bufs=3, space="PSUM"))
        psum_t_pool = ctx.enter_context(tc.tile_pool(name="pst", bufs=2, space="PSUM"))
        m_pool = ctx.enter_context(tc.tile_pool(name="m", bufs=1))
        tmp_pool = ctx.enter_context(tc.tile_pool(name="tmp", bufs=2))
        out_pool = ctx.enter_context(tc.tile_pool(name="outp", bufs=3))

        # Small consts first so the first matmul group's weights land early,
        # then the one-hot stream in word order.
        wta_t = const_pool.tile([128, NKDK, 2, EMB], F8)
        nc.sync.dma_start(
            wta_t[:].rearrange("p a b c -> p (a b c)"), wta_d[:])
        bias_t = const_pool.tile([128, 6], F32)
        nc.sync.dma_start(bias_t[:], bias_d[:])
        ident = const_pool.tile([128, 128], F32)
        nc.sync.dma_start(ident[:], id_d[:])
        mask_t = const_pool.tile([128, nwb], F32)
        nc.sync.dma_start(mask_t[:], mask_d[:])

        oh_t = const_pool.tile([128, 2, words * L], F8)
        assert words % OH_CHUNK == 0
        for c in range(words // OH_CHUNK):
            sl = slice(c * OH_CHUNK * L, (c + 1) * OH_CHUNK * L)
            nc.sync.dma_start(oh_t[:, :, sl], oh_d[:, :, sl])

        M = {}
        for ki in range(3):
            for oc in range(2):
                M[(ki, oc)] = m_pool.tile(
                    [128, words], F32, tag=f"m{ki}{oc}", name=f"m{ki}{oc}")
        C = [m_pool.tile([128, words], F32, tag=f"c{oc}", name=f"c{oc}")
             for oc in range(2)]

        # PE warm-up (HAM clock ramp) while the first input chunks drain
        scratch = const_pool.tile([128, 512], BF16)
        nc.vector.memset(scratch[:], 0.0)
        warm = psum_pool.tile([128, 512], F32, tag="ps0")
        for _ in range(WARMUP_MM):
            nc.tensor.matmul(warm[:], scratch[:, :128], scratch[:],
                             start=True, stop=True)

        oh_v = oh_t[:].rearrange("p i (w t) -> p i w t", t=L)

        def do_bank(ki, w0, halves, nw):
            k = KS[ki]
            lk = L - k + 1
            for oc in range(2):
                ps = psum_pool.tile([128, halves, nw, lk], F32,
                                    tag=f"ps{oc}", name=f"ps{ki}{oc}")
                for h in range(halves):
                    for dk in range(k):
                        nc.tensor.matmul(
                            ps[:, h],
                            wta_t[:, _kdk_off(ki, dk), :,
                                  oc * 128:(oc + 1) * 128],
                            oh_v[:, :, w0 + h * nw:w0 + (h + 1) * nw,
                                 dk:dk + lk],
                            start=(dk == 0), stop=(dk == k - 1),
                            perf_mode=mybir.MatmulPerfMode.DoubleRow,
                        )
                nc.vector.reduce_max(
                    M[(ki, oc)][:, w0:w0 + halves * nw]
                    .rearrange("p (h w) -> p h w", h=halves),
                    ps[:], axis=mybir.AxisListType.X)

        wb_done = 0
        covered = 0

        def combine(hi):
            """Fold M into C for columns [covered, hi): bias adds on ScalarE,
            maxes on DVE."""
            nonlocal covered
            sl = slice(covered, hi)
            n = hi - covered
            for oc in range(2):
                t4 = tmp_pool.tile([128, n], F32, tag="t4", name="t4")
                nc.scalar.add(
                    C[oc][:, sl], M[(0, oc)][:, sl],
                    bias_t[:, 3 * oc:3 * oc + 1])
                nc.scalar.add(
                    t4[:], M[(1, oc)][:, sl], bias_t[:, 3 * oc + 1:3 * oc + 2])
                nc.vector.tensor_max(C[oc][:, sl], C[oc][:, sl], t4[:])
                nc.scalar.add(
                    t4[:], M[(2, oc)][:, sl], bias_t[:, 3 * oc + 2:3 * oc + 3])
                nc.vector.tensor_max(C[oc][:, sl], C[oc][:, sl], t4[:])
            covered = hi

        def emit_ready():
            """Emit finished 128-word output blocks (transpose + relu*mask).
            Called at round start so the PE transpose doesn't head-of-line-
            block the matmul stream on the DVE combine."""
            nonlocal wb_done
            while (wb_done + 1) * 128 <= covered:
                wb = wb_done
                for oc in range(2):
                    pst = psum_t_pool.tile([128, 128], F32, tag="pst",
                                           name="pst")
                    nc.tensor.transpose(
                        pst[:], C[oc][:, wb * 128:(wb + 1) * 128], ident[:])
                    ot = out_pool.tile([128, 128], F32, tag="ot", name="ot")
                    nc.scalar.activation(
                        ot[:], pst[:], mybir.ActivationFunctionType.Relu,
                        scale=mask_t[:, wb:wb + 1])
                    nc.sync.dma_start(
                        out_d[wb * 128:(wb + 1) * 128,
                              oc * 128:(oc + 1) * 128], ot[:])
                wb_done += 1

        idx = [0, 0, 0]
        cov = [0, 0, 0]
        while any(idx[ki] < len(plans[ki]) for ki in range(3)):
            emit_ready()
            for ki in range(3):
                if idx[ki] < len(plans[ki]):
                    w0, halves, nw = plans[ki][idx[ki]]
                    do_bank(ki, w0, halves, nw)
                    idx[ki] += 1
                    cov[ki] = w0 + halves * nw
            new_cov = min(cov)
            if new_cov > covered:
                combine(new_cov)
        emit_ready()
        assert covered == words and wb_done == nwb, (covered, wb_done)

    nc.compile()
    return nc


def prep_shared(emb, w3, w4, w5, b3, b4, b5):
    emb64 = np.asarray(emb, np.float64)
    wta = np.empty((128, NKDK, 2, EMB), dtype=np.float32)
    for ki, w in enumerate((w3, w4, w5)):
        k = KS[ki]
        w64 = np.asarray(w, np.float64)
        for dk in range(k):
            t = (emb64 @ w64[:, :, dk].T) * SCALE      # [256 char, 256 out]
            off = _kdk_off(ki, dk)
            wta[:, off, 0, :] = t[:128]
            wta[:, off, 1, :] = t[128:]
    np.clip(wta, -240.0, 240.0, out=wta)
    wta8 = wta.reshape(128, -1).astype(ml_dtypes.float8_e4m3fn)
    bias = np.empty((128, 6), dtype=np.float32)
    for oc in range(2):
        for ki, b in enumerate((b3, b4, b5)):
            bias[:, 3 * oc + ki] = b[oc * 128:(oc + 1) * 128] * SCALE
    ident = np.eye(128, dtype=np.float32)
    return wta8, bias, ident


def prep_core(xf, lensf, words=W):
    """Per-core one-hot + mask packing. xf: [words, L] int32, lensf: [words].
    oh[p, i, w*L+t] = (x[w, t] == i*128 + p), as raw e4m3 bytes (1.0 = 0x38).
    """
    xi = xf.reshape(-1).astype(np.int64)
    oh = np.zeros((128, 2, words * L), dtype=np.uint8)
    oh[xi % 128, xi >> 7, np.arange(words * L)] = 0x38
    nwb = words // 128
    maskp = (lensf.reshape(nwb, 128).T != 0).astype(np.float32) * (1.0 / SCALE)
    return oh.view(ml_dtypes.float8_e4m3fn), np.ascontiguousarray(maskp)


_CACHE = {}


def _get_nc(words=W):
    if words not in _CACHE:
        _CACHE[words] = build_bass(words)
    return _CACHE[words]


def run(x, lens, emb, w3, b3, w4, b4, w5, b5, trace=False, **spmd_kwargs):
    x = np.asarray(x)
    lens = np.asarray(lens)
    nc = _get_nc()
    wta8, bias, ident = prep_shared(
        np.asarray(emb), np.asarray(w3), np.asarray(w4), np.asarray(w5),
        np.asarray(b3), np.asarray(b4), np.asarray(b5))
    xf = x.reshape(B * S, L)
    lensf = lens.reshape(B * S)
    in_maps = []
    for c in range(NCORES):
        sl = slice(c * W, (c + 1) * W)
        oh, maskp = prep_core(xf[sl], lensf[sl])
        in_maps.append({
            "oh": oh, "wta": wta8, "bias": bias, "maskp": maskp,
            "ident": ident,
        })
    res = run_bass_kernel_spmd(
        nc, in_maps, core_ids=list(range(NCORES)), trace=trace, **spmd_kwargs)
    out = np.concatenate([r["out"] for r in res.results], axis=0)
    return np.ascontiguousarray(out.reshape(B, S, EMB).astype(np.float32)), res


def kernel(x, lens, emb, w3, b3, w4, b4, w5, b5, **unused):
    out, _ = run(x, lens, emb, w3, b3, w4, b4, w5, b5)
    return out
